# revision 1
# baseline (speedup 1.0000x reference)
"""Trainium2 Bass kernel for block-diagonal complex matmul (ComplexMult).

Reference semantics (per block k, complex):
    out[o, x, y] = sum_i inp[i, x, y] * weight[i, o] + bias[o]
with inp/weight/bias stored as interleaved (real, imag) in the last dim.

Sharding: NUM_BLOCKS == 8 == n_cores -> block k runs on core k (fully
data-parallel, no collectives).

Per-core kernel: DMAs move the (r, i)-interleaved data contiguously in
both directions.  fp32r matmuls require a contiguous PSUM destination
(walrus checkMatmultFP32r: innermost dst step must be 1 with even count;
the *moving* operand may be strided with even count).  So each group of
1024 interleaved fp32 columns (512 complex points) is computed
de-interleaved in PSUM — bank R holds real, bank I holds imag:
  MM1: ps[0:512]    = wr.T @ ar   (moving = even cols, start clears bank R)
  MM2: ps[512:1024] = wr.T @ ai   (moving = odd cols,  start clears bank I)
  MM3: ps[0:512]   += -wi.T @ ai  (accumulate)
  MM4: ps[512:1024] += wi.T @ ar  (accumulate)
The DVE eviction re-interleaves and adds bias in the same op:
tensor_scalar_add with a per-partition [96,1] bias operand, reading the
contiguous PSUM half and writing the stride-2 column slice of the SBUF
out tile.  All HBM DMAs stay fully contiguous.
"""

import numpy as np
from contextlib import ExitStack

NUM_BLOCKS = 8
BLOCK = 96            # i == o == 96
H, W = 360, 181
N_SP = H * W          # complex points per block
N_COLS = N_SP * 2     # fp32 columns per block (interleaved r,i) = 130320
TILE_COLS = 4096      # fp32 columns per DMA tile (16 KiB per partition)
GROUP = 1024          # fp32 columns per PSUM group (2 banks: real | imag)

_cache = {}


def _patched_drain_and_barrier(self, tick_clock, wait_clock):
    """TileContext._drain_and_barrier emits a kernel-tail drain carrying one
    sync wait per outstanding semaphore, but walrus only encodes ONE wait per
    instruction.  Keep one wait on the drain and re-emit the rest as
    standalone single-wait SP instructions."""
    import bass_rust as _br
    from concourse.vector_clock import ScopedClock

    drain_inst = self.nc.sync.drain()
    wait_clock.add_sem_waits(
        drain_inst.ins, ScopedClock({None: tick_clock.global_clock}))
    ins = drain_inst.ins
    si = ins.sync_info
    waits = list(si.on_wait) if si is not None else []
    assert self.sems is not None
    popped = self.nc._tile_sem_poison_stack.pop()
    assert popped is self._sem_poison
    if len(waits) > 1:
        ins.sync_info = _br.SyncInfo(on_wait=[waits[0]],
                                     on_update=list(si.on_update))
        by_name = {h.name: h for h in self.sems.allocated().values()}
        for w in waits[1:]:
            self.nc.sync.wait_ge(by_name[w.ant_name], w.wait_value)
    self.nc.all_engine_barrier()
    self.nc.clear_and_free_semaphores(list(self.sems.allocated().values()))
    self.nc.all_engine_barrier()


def _make_patched_lower(orig_lower):
    def _patched_lower(self, ordered):
        """Walrus encodes at most ONE sync wait per instruction.  Split any
        multi-wait instruction: excess waits become standalone
        InstEventSemaphore carriers on the same engine, inserted before it."""
        import bass_rust as _br
        import concourse.mybir as mybir

        for bb, insts in list(ordered.items()):
            out = []
            for inst in insts:
                si = inst.sync_info
                waits = list(si.on_wait) if si is not None else []
                if len(waits) > 1:
                    for w in waits[:-1]:
                        ev = mybir.InstEventSemaphore(
                            name=self.nc.get_next_instruction_name())
                        ev.engine = inst.engine
                        ev.sync_info = _br.SyncInfo(on_wait=[w], on_update=[])
                        out.append(ev)
                    inst.sync_info = _br.SyncInfo(
                        on_wait=[waits[-1]], on_update=list(si.on_update))
                out.append(inst)
            ordered[bb] = out
        return orig_lower(self, ordered)
    return _patched_lower


def _build(n_cols=N_COLS, use_f32r=True):
    import concourse.bass as bass
    import concourse.mybir as mybir
    import concourse.tile as tile

    tile.TileContext._drain_and_barrier = _patched_drain_and_barrier
    if not getattr(tile.TileContext, "_ant_lower_patched", False):
        tile.TileContext._lower_ordered_insts = _make_patched_lower(
            tile.TileContext._lower_ordered_insts)
        tile.TileContext._ant_lower_patched = True

    nc = bass.Bass(trn_type="TRN2", debug=False)
    f32 = mybir.dt.float32
    mm_dt = mybir.dt.float32r if use_f32r else mybir.dt.float32

    # wgt3 is host-prepared as [wr | -wi | wi] so no on-device negation is
    # needed and the fp32r stationaries come straight off one DMA (the BIR
    # verifier requires every fp32r matmul operand's producer to emit fp32r).
    inp = nc.dram_tensor("inp", [BLOCK, n_cols], f32, kind="ExternalInput").ap()
    wgt3 = nc.dram_tensor("wgt3", [BLOCK, 3 * BLOCK], f32,
                          kind="ExternalInput").ap()
    bia = nc.dram_tensor("bia", [BLOCK, 2], f32, kind="ExternalInput").ap()
    out = nc.dram_tensor("out", [BLOCK, n_cols], f32, kind="ExternalOutput").ap()

    with tile.TileContext(nc) as tc, ExitStack() as ctx:
        const = ctx.enter_context(tc.tile_pool(name="const", bufs=1))
        inpool = ctx.enter_context(tc.tile_pool(name="inpool", bufs=4))
        outpool = ctx.enter_context(tc.tile_pool(name="outpool", bufs=4))
        psums = ctx.enter_context(tc.tile_pool(name="psums", bufs=3, space="PSUM"))
        scr = ctx.enter_context(tc.tile_pool(name="scr", bufs=1, space="PSUM"))
        scratch = scr.tile([1, GROUP // 2], mybir.dt.float32)

        wmat = const.tile([BLOCK, 3 * BLOCK], mm_dt)
        nc.sync.dma_start(wmat[:, :], wgt3[:, :].bitcast(mm_dt))
        bias2 = const.tile([BLOCK, 2], f32)
        nc.sync.dma_start(bias2[:, :], bia[:, :])
        # brep = [bias_r x512 | bias_i x512], matching the de-interleaved
        # PSUM group layout, so one tensor_tensor evicts a whole group.
        brep = const.tile([BLOCK, GROUP], f32)
        nc.vector.tensor_copy(brep[:, 0:1], bias2[:, 0:1])
        nc.vector.tensor_copy(brep[:, GROUP // 2:GROUP // 2 + 1], bias2[:, 1:2])
        w = 1
        while w < GROUP // 2:
            nc.vector.tensor_copy(brep[:, w:2 * w], brep[:, 0:w])
            nc.vector.tensor_copy(brep[:, GROUP // 2 + w:GROUP // 2 + 2 * w],
                                  brep[:, GROUP // 2:GROUP // 2 + w])
            w *= 2

        wr_m = wmat[:, 0:BLOCK]
        nwi_m = wmat[:, BLOCK:2 * BLOCK]
        wi_m = wmat[:, 2 * BLOCK:3 * BLOCK]

        # PE prologue burst while the first input DMAs are in flight: absorbs
        # the wmat-DMA wait ahead of the steady-state groups and measured
        # fastest at this length (302-308us vs 310us with a short prologue;
        # MM duration itself is set by the stride-2 moving reads, not HAM).
        for _ in range(16):
            nc.tensor.matmul(scratch[:, 0:3 * BLOCK], wmat[:, 0:1],
                             wmat[:, :], start=True, stop=True,
                             skip_group_check=True)

        # Tapered tiling: small tiles at the start (compute starts sooner)
        # and at the end (shorter pipeline drain), 4 KiB-col tiles between.
        # Ragged trailing group is fine: matmul dsts stay contiguous/even.
        assert n_cols % 2 == 0
        ranges = []
        c = 0
        taper = TILE_COLS // 2
        if n_cols > 2 * TILE_COLS + 2 * taper:
            ranges += [(0, taper), (taper, 2 * taper)]
            c = 2 * taper
            while n_cols - c > TILE_COLS + 2 * taper:
                ranges.append((c, c + TILE_COLS))
                c += TILE_COLS
            mid = c + (n_cols - c) // 2
            mid += mid % 2
            ranges += [(c, mid), (mid, n_cols)]
        else:
            while c < n_cols:
                e = min(n_cols, c + TILE_COLS)
                ranges.append((c, e))
                c = e
        half = GROUP // 2  # 512: one PSUM bank, also the fp32 matmul max N
        for jt, (c0, c1) in enumerate(ranges):
            cols = c1 - c0
            tin = inpool.tile([BLOCK, cols], mm_dt, tag="tin")
            # Alternate the input ring between sync (HWDGE) and gpsimd
            # (SWDGE) so SDMA engines have 3 descriptor rings to drain and
            # starve less at packet boundaries.
            in_eng = nc.sync if jt % 2 == 0 else nc.gpsimd
            in_eng.dma_start(tin[:, :], inp[:, c0:c1].bitcast(mm_dt))
            tout = outpool.tile([BLOCK, cols], f32, tag="tout")
            for g0 in range(0, cols, GROUP):
                gc = min(GROUP, cols - g0)
                gh = gc // 2
                mv_even = tin[:, g0:g0 + gc:2]      # ar
                mv_odd = tin[:, g0 + 1:g0 + gc:2]   # ai
                ps = psums.tile([BLOCK, GROUP], f32, tag="ps")
                # wr twice first (stationary reuse), then the cross terms
                nc.tensor.matmul(ps[:, 0:gh], wr_m, mv_even,
                                 start=True, stop=False)
                nc.tensor.matmul(ps[:, half:half + gh], wr_m, mv_odd,
                                 start=True, stop=False)
                nc.tensor.matmul(ps[:, 0:gh], nwi_m, mv_odd,
                                 start=False, stop=True)
                nc.tensor.matmul(ps[:, half:half + gh], wi_m, mv_even,
                                 start=False, stop=True)
                # One DVE op per group: add bias and re-interleave.
                # out iterates (c, n) -> address g0 + 2n + c, matching the
                # (real block | imag block) order of ps/brep.
                out_ap = tout[:, g0:g0 + gc].rearrange("p (n c) -> p c n", c=2)
                ps_ap = ps[:, :].rearrange("p (c n) -> p c n", c=2)[:, :, 0:gh]
                brep_ap = brep[:, :].rearrange("p (c n) -> p c n", c=2)[:, :, 0:gh]
                nc.vector.tensor_add(out_ap, ps_ap, brep_ap)
            # out-DMAs go via the scalar engine's HWDGE ring so reads
            # (sync ring) and writes don't share one queue.
            nc.scalar.dma_start(out[:, c0:c1], tout[:, :])
    return nc


def _get_nc(n_cols=N_COLS, use_f32r=True):
    key = (n_cols, use_f32r)
    if key not in _cache:
        _cache[key] = _build(n_cols, use_f32r)
    return _cache[key]


TRACE = False        # set True (e.g. from test.py) to capture an NTFF profile
TRACE_DIR = None     # optional dir for NTFF/perfetto artifacts when TRACE
LAST_RESULTS = None  # BassKernelResults of the most recent kernel() call


def kernel(inp, weight, bias):
    """inp [1,8,96,360,181,2] f32, weight [8,96,96,2], bias [8,96,1,1,2]
    -> [1,8,96,360,181,2] f32."""
    global LAST_RESULTS
    from concourse.bass_utils import run_bass_kernel_spmd

    nc = _get_nc()
    in_maps = []
    for k in range(NUM_BLOCKS):
        wk = weight[k].astype(np.float32, copy=False)
        wgt3 = np.concatenate([wk[:, :, 0], -wk[:, :, 1], wk[:, :, 1]], axis=1)
        in_maps.append({
            "inp": np.ascontiguousarray(
                inp[0, k].reshape(BLOCK, N_COLS).astype(np.float32, copy=False)),
            "wgt3": np.ascontiguousarray(wgt3),
            "bia": np.ascontiguousarray(
                bias[k, :, 0, 0, :].astype(np.float32, copy=False)),
        })
    res = run_bass_kernel_spmd(nc, in_maps, list(range(NUM_BLOCKS)),
                               trace=TRACE, tmpdir=TRACE_DIR)
    LAST_RESULTS = res
    outs = [res.results[k]["out"].reshape(BLOCK, H, W, 2)
            for k in range(NUM_BLOCKS)]
    return np.stack(outs, axis=0)[None].astype(np.float32, copy=False)

